# revision 16
# baseline (speedup 1.0000x reference)
"""GQA kernel for Trainium2 (Bass/Tile), 8-core head-parallel. v4.

Problem: x(1,2048,1024), Wq(1024,1024)+bq, Wk/Wv(1024,256)+bk/bv,
16 Q heads / 4 KV heads, head_dim 64, full (non-causal) softmax attention.
Reference output is attn(B,H,S,Dh) reshaped DIRECTLY to (B,S,H*Dh):
core d owns Q heads {2d, 2d+1} (one KV group d//2), producing a contiguous
(256,1024) slab of the final output; gather = concat.

The kernel is ACT-bound: 64 exp instructions of [128,1024] (~1.11us each)
are the critical path. Scheduling keeps the Scalar engine busy on exp from
~6us onward:
- fp16 everywhere (same PE speed as bf16, 10x better accuracy).
- flat software-pipelined item list: item = one attention round
  (2 scores mm -> exp -> 2 PV mm). S(i+1) is emitted before PV(i); a
  3-deep scores ring keeps S(i+2) off the exp critical path.
- first 16 items interleave groups (0,0) and (1,0) so early rounds only
  need early xT blocks while qb2/qb3 still stream from HBM.
- xt DRAM layout is [p][qb][c][s-block] so each xT block DMA is a single
  contiguous descriptor per partition (HWDGE trigger is cheap); weights
  ride the Activation engine's HWDGE queue, parallel to the sync queue.
- KV/Q projections and V' transposes are fillers inside the DMA-gated
  fill window, placed just-in-time before their first consumer.
- output drain avoids PSUM/PE entirely: reciprocal of the denominator row
  (DVE) -> partition_broadcast (Pool) -> tensor_tensor multiply (DVE) ->
  4 DMA-XBAR transposes (sync HWDGE) -> fp16 output DMA (host casts).
- PSUM: 3x scores[128,1024] (tag also hosts proj + V'-transpose tiles)
  + 2x pso[65,512] = 8 banks.
"""

import numpy as np

import concourse.bass as bass
import concourse.mybir as mybir
import concourse.tile as tile
from concourse import bacc
from concourse.bass_utils import run_bass_kernel_spmd
from concourse.masks import make_identity

F32 = mybir.dt.float32
F16 = mybir.dt.float16
AF = mybir.ActivationFunctionType
MULT = mybir.AluOpType.mult

S = 2048
DIM = 1024
HD = 64          # head dim
N_CORES = 8
NCH = DIM // 128  # 8 contraction chunks of 128


def build_kernel():
    nc = bacc.Bacc("TRN2", target_bir_lowering=False, debug=False, num_devices=N_CORES)

    xt_d = nc.dram_tensor("xt", [128, 4, NCH, 512], F16, kind="ExternalInput").ap()
    wq_d = nc.dram_tensor("wq", [DIM, 128], F16, kind="ExternalInput").ap()
    bq_d = nc.dram_tensor("bq", [128, 1], F32, kind="ExternalInput").ap()
    wkv_d = nc.dram_tensor("wkv", [DIM, 128], F16, kind="ExternalInput").ap()
    bkv_d = nc.dram_tensor("bkv", [128, 1], F32, kind="ExternalInput").ap()
    o_d = nc.dram_tensor("o", [2, S, HD], F16, kind="ExternalOutput").ap()

    with tile.TileContext(nc) as tc:
        with (
            tc.tile_pool(name="const", bufs=1) as const_pool,
            tc.tile_pool(name="persist", bufs=1) as persist_pool,
            tc.tile_pool(name="pt", bufs=3) as pt_pool,
            tc.tile_pool(name="outs", bufs=2) as out_pool,
            tc.tile_pool(name="ps_big", bufs=3, space="PSUM") as ps_big,
            tc.tile_pool(name="ps_o", bufs=2, space="PSUM") as ps_o,
        ):
            # ---- x stream: one single-descriptor DMA per block (sync HWDGE);
            # block qb0 split in halves so the KV projection starts sooner
            xT = persist_pool.tile([128, 4, NCH, 512], F16)   # 4 MB
            nc.sync.dma_start(xT[:, 0, 0:4, :], xt_d[:, 0, 0:4, :])
            nc.sync.dma_start(xT[:, 0, 4:8, :], xt_d[:, 0, 4:8, :])
            for qb in range(1, 4):
                nc.sync.dma_start(xT[:, qb], xt_d[:, qb])

            # ---- weights + biases on the Activation HWDGE queue (runs before
            # any exp), transfers overlap the sync queue's x stream
            wkv_sb = const_pool.tile([128, NCH, 128], F16)
            wq_sb = const_pool.tile([128, NCH, 128], F16)
            bq_sb = const_pool.tile([128, 1], F32)
            bkv_sb = const_pool.tile([128, 1], F32)
            nc.scalar.dma_start(wkv_sb[:], wkv_d.rearrange("(c p) d -> p c d", p=128))
            nc.scalar.dma_start(wq_sb[:], wq_d.rearrange("(c p) d -> p c d", p=128))
            nc.scalar.dma_start(bkv_sb[:], bkv_d[:])
            nc.scalar.dma_start(bq_sb[:], bq_d[:])

            # ---- small constants (gpsimd software queue) ----
            ident32 = const_pool.tile([128, 128], F32)
            make_identity(nc, ident32[:])
            # I_64 in partitions 64..127 for base-64 V transposes
            ident2 = const_pool.tile([128, 64], F16)
            nc.vector.tensor_copy(ident2[0:64, :], ident32[0:64, 0:64])
            nc.gpsimd.dma_start(ident2[64:128, :], ident2[0:64, :])

            # ---- persistent SBUF tensors ----
            qt_sb = persist_pool.tile([128, S], F16)       # heads packed: h*64+d
            kv_sb = persist_pool.tile([128, S], F16)       # rows 0:64 KT, 64:128 VT
            kt2 = persist_pool.tile([128, S], F16)         # rows 64:128 = KT (for h1)
            v_sb = persist_pool.tile([128, 16, 65], F16)   # V' chunks (+ones col)
            nc.gpsimd.memset(v_sb[:, :, 64:65], 1.0)

            # ---- projection / setup helpers ----
            proj_tiles = {}

            def proj_chunks(w_sb, key, qb, cs):
                if cs[0] == 0:
                    proj_tiles[key] = ps_big.tile(
                        [128, 512], F32, tag="big", name="proj")
                pt_ = proj_tiles[key]
                for c in cs:
                    nc.tensor.matmul(pt_[:], w_sb[:, c, :], xT[:, qb, c, :],
                                     start=(c == 0), stop=(c == NCH - 1))

            def kv_drain(key, qb):
                sl = slice(qb * 512, (qb + 1) * 512)
                nc.vector.tensor_scalar_add(kv_sb[:, sl], proj_tiles[key][:],
                                            bkv_sb[:])
                # KT rows for head 1 live at partitions 64.. (DMA shifts partitions)
                nc.gpsimd.dma_start(kt2[64:128, sl], kv_sb[0:64, sl])

            def q_drain(key, qb):
                sl = slice(qb * 512, (qb + 1) * 512)
                nc.vector.tensor_scalar_add(qt_sb[:, sl], proj_tiles[key][:],
                                            bq_sb[:])

            def v_transpose4(kb0):
                """Transpose V chunks kb0..kb0+3 (one PSUM tile, one DVE copy)."""
                vt = ps_big.tile([128, 4, 66], F16, tag="big", name="vt")
                for j in range(4):
                    kb = kb0 + j
                    nc.tensor.matmul(
                        vt[:, j, 0:64], kv_sb[64:128, kb * 128:(kb + 1) * 128],
                        ident2[64:128, :], is_transpose=True)
                nc.vector.tensor_copy(v_sb[:, kb0:kb0 + 4, 0:64], vt[:, :, 0:64])

            # ---- attention round machinery ----
            class Group:
                def __init__(self, h, qb):
                    self.h, self.qb = h, qb
                    self.hb = h * HD
                    self.qsl = slice(qb * 512, (qb + 1) * 512)
                    self.pso = None
                    self.pss = {}
                    self.pts = {}

            def scores_exp(grp, r):
                pss = ps_big.tile([128, 1024], F32, tag="big", name="pss")
                grp.pss[r] = pss
                for u in range(2):
                    kb = r * 2 + u
                    if grp.h == 0:
                        lhs = kv_sb[0:64, kb * 128:(kb + 1) * 128]
                    else:
                        lhs = kt2[64:128, kb * 128:(kb + 1) * 128]
                    nc.tensor.matmul(
                        pss[:, u * 512:(u + 1) * 512], lhs,
                        qt_sb[grp.hb:grp.hb + HD, grp.qsl], start=True, stop=True)
                ptile = pt_pool.tile([128, 1024], F16, name="pt")
                grp.pts[r] = ptile
                nc.scalar.activation(ptile[:], pss[:], AF.Exp)

            def pv(grp, r):
                if grp.pso is None:
                    grp.pso = ps_o.tile([65, 512], F32, tag="ot", name="pso")
                for u in range(2):
                    kb = r * 2 + u
                    nc.tensor.matmul(
                        grp.pso[:], v_sb[:, kb, :],
                        grp.pts[r][:, u * 512:(u + 1) * 512],
                        start=(kb == 0), stop=(kb == 15),
                        skip_group_check=True)
                del grp.pts[r], grp.pss[r]

            def drain(grp):
                # normalize + transpose without touching PE or extra PSUM:
                # recip of denominator row, broadcast across partitions,
                # multiply O^T rows, then 4 DMA-XBAR transposes to s-major.
                rcp_row = out_pool.tile([1, 512], F16, tag="rcp_row")
                with nc.allow_low_precision(reason="softmax denom recip in fp16"):
                    nc.vector.reciprocal(rcp_row[:], grp.pso[64:65, :])
                rcp64 = out_pool.tile([64, 512], F16, tag="rcp64")
                nc.gpsimd.partition_broadcast(rcp64[:], rcp_row[:])
                o_norm = out_pool.tile([64, 512], F16, tag="o_norm")
                nc.vector.tensor_tensor(o_norm[:], grp.pso[0:64, :], rcp64[:], MULT)
                o_t = out_pool.tile([128, 4, HD], F16, tag="o_t")
                for j in range(4):
                    nc.sync.dma_start(o_t[:, j, :],
                                      o_norm[:, j * 128:(j + 1) * 128],
                                      transpose=True)
                nc.gpsimd.dma_start(
                    o_d[grp.h, grp.qsl, :].rearrange("(t j) c -> j t c", j=128),
                    o_t[:])

            # ---- pre-fill: KV(qb0) chasing the two half-DMAs, V'(0..3), Q(qb0)
            proj_chunks(wkv_sb, "kv0", 0, [0, 1, 2, 3])
            proj_chunks(wkv_sb, "kv0", 0, [4, 5, 6, 7])
            kv_drain("kv0", 0)
            v_transpose4(0)
            proj_chunks(wq_sb, "q0", 0, list(range(NCH)))
            q_drain("q0", 0)

            # ---- item list: (group, round); A/B interleaved to track the
            # xT DMA stream, then the remaining 6 groups sequential ----
            A = Group(0, 0)
            B = Group(1, 0)
            C, D, E, F, G, H = (Group(0, 1), Group(1, 1), Group(0, 2),
                                Group(1, 2), Group(0, 3), Group(1, 3))
            items = ([(A, 0), (A, 1), (B, 0), (B, 1), (A, 2), (A, 3),
                      (B, 2), (B, 3), (A, 4), (A, 5), (B, 4), (B, 5),
                      (A, 6), (A, 7), (B, 6), (B, 7)]
                     + [(grp, r) for grp in (C, D, E, F, G, H)
                        for r in range(8)])

            # fillers: pre[i] runs before S(item i) (producers, program order
            # = dataflow). All projection work lands in the DMA-gated fill.
            pre = {i: [] for i in range(len(items))}
            pre[3].append(lambda: proj_chunks(wkv_sb, "kv1", 1, [0, 1, 2, 3]))
            pre[4].append(lambda: (proj_chunks(wkv_sb, "kv1", 1, [4, 5, 6, 7]),
                                   kv_drain("kv1", 1)))
            pre[5].append(lambda: (v_transpose4(4),
                                   proj_chunks(wq_sb, "q1", 1, [0, 1, 2, 3])))
            pre[6].append(lambda: (proj_chunks(wq_sb, "q1", 1, [4, 5, 6, 7]),
                                   q_drain("q1", 1)))
            pre[7].append(lambda: proj_chunks(wkv_sb, "kv2", 2, [0, 1, 2, 3]))
            pre[8].append(lambda: (proj_chunks(wkv_sb, "kv2", 2, [4, 5, 6, 7]),
                                   kv_drain("kv2", 2)))
            pre[9].append(lambda: (v_transpose4(8),
                                   proj_chunks(wq_sb, "q2", 2, [0, 1, 2, 3])))
            pre[10].append(lambda: (proj_chunks(wq_sb, "q2", 2, [4, 5, 6, 7]),
                                    q_drain("q2", 2)))
            pre[11].append(lambda: proj_chunks(wkv_sb, "kv3", 3, [0, 1, 2, 3]))
            pre[12].append(lambda: (proj_chunks(wkv_sb, "kv3", 3, [4, 5, 6, 7]),
                                    kv_drain("kv3", 3)))
            pre[13].append(lambda: (v_transpose4(12),
                                    proj_chunks(wq_sb, "q3", 3, [0, 1, 2, 3])))
            pre[14].append(lambda: (proj_chunks(wq_sb, "q3", 3, [4, 5, 6, 7]),
                                    q_drain("q3", 3)))

            # ---- flat software pipeline ----
            n = len(items)
            for f in pre[0]:
                f()
            scores_exp(*items[0])
            for i in range(n):
                if i + 1 < n:
                    for f in pre[i + 1]:
                        f()
                    scores_exp(*items[i + 1])
                grp, r = items[i]
                pv(grp, r)
                if r == 7:
                    drain(grp)

    nc.compile()
    return nc


_NC_CACHE = None


def make_in_maps(inputs):
    x = np.asarray(inputs["x"], np.float32).reshape(S, DIM)
    # [dim, s] -> [p][qb][c][t] with dim = c*128+p, s = qb*512+t
    xt = np.ascontiguousarray(
        x.T.reshape(NCH, 128, 4, 512).transpose(1, 2, 0, 3)).astype(np.float16)
    Wq = np.asarray(inputs["Wq"], np.float32)
    bq = np.asarray(inputs["bq"], np.float32)
    Wk = np.asarray(inputs["Wk"], np.float32)
    bk = np.asarray(inputs["bk"], np.float32)
    Wv = np.asarray(inputs["Wv"], np.float32)
    bv = np.asarray(inputs["bv"], np.float32)

    in_maps = []
    for d in range(N_CORES):
        g = d // 2
        wkv = np.concatenate(
            [Wk[:, g * 64:(g + 1) * 64], Wv[:, g * 64:(g + 1) * 64]], axis=1)
        bkv = np.concatenate([bk[g * 64:(g + 1) * 64], bv[g * 64:(g + 1) * 64]])
        in_maps.append({
            "xt": xt,
            "wq": (np.ascontiguousarray(Wq[:, d * 128:(d + 1) * 128]) / 8.0
                   ).astype(np.float16),
            "bq": (bq[d * 128:(d + 1) * 128] / 8.0).reshape(128, 1),
            "wkv": np.ascontiguousarray(wkv).astype(np.float16),
            "bkv": bkv.reshape(128, 1).copy(),
        })
    return in_maps


def kernel(**inputs) -> np.ndarray:
    global _NC_CACHE
    if _NC_CACHE is None:
        _NC_CACHE = build_kernel()
    nc = _NC_CACHE
    in_maps = make_in_maps(inputs)
    res = run_bass_kernel_spmd(nc, in_maps, list(range(N_CORES)))
    blocks = [np.asarray(res.results[d]["o"]).astype(np.float32).reshape(256, DIM)
              for d in range(N_CORES)]
    return np.concatenate(blocks, axis=0).reshape(1, S, DIM)


# revision 20
# speedup vs baseline: 1.3443x; 1.3443x over previous
"""GQA kernel for Trainium2 (Bass/Tile), 8-core head-parallel. v4.

Problem: x(1,2048,1024), Wq(1024,1024)+bq, Wk/Wv(1024,256)+bk/bv,
16 Q heads / 4 KV heads, head_dim 64, full (non-causal) softmax attention.
Reference output is attn(B,H,S,Dh) reshaped DIRECTLY to (B,S,H*Dh):
core d owns Q heads {2d, 2d+1} (one KV group d//2), producing a contiguous
(256,1024) slab of the final output; gather = concat.

The kernel is ACT-bound: 64 exp instructions of [128,1024] (~1.11us each)
are the critical path. Scheduling keeps the Scalar engine busy on exp from
~6us onward:
- fp16 everywhere (same PE speed as bf16, 10x better accuracy).
- flat software-pipelined item list: item = one attention round
  (2 scores mm -> exp -> 2 PV mm). S(i+1) is emitted before PV(i); a
  3-deep scores ring keeps S(i+2) off the exp critical path.
- first 16 items interleave groups (0,0) and (1,0) so early rounds only
  need early xT blocks while qb2/qb3 still stream from HBM.
- xt DRAM layout is [p][qb][c][s-block] so each xT block DMA is a single
  contiguous descriptor per partition (HWDGE trigger is cheap); weights
  ride the Activation engine's HWDGE queue, parallel to the sync queue.
- KV/Q projections and V' transposes are fillers inside the DMA-gated
  fill window, placed just-in-time before their first consumer.
- output drain avoids PSUM/PE entirely: reciprocal of the denominator row
  (DVE) -> partition_broadcast (Pool) -> tensor_tensor multiply (DVE) ->
  4 DMA-XBAR transposes (sync HWDGE) -> fp16 output DMA (host casts).
- PSUM: 3x scores[128,1024] (tag also hosts proj + V'-transpose tiles)
  + 2x pso[65,512] = 8 banks.
"""

import numpy as np

import concourse.bass as bass
import concourse.mybir as mybir
import concourse.tile as tile
from concourse import bacc
from concourse.bass_utils import run_bass_kernel_spmd
from concourse.masks import make_identity

F32 = mybir.dt.float32
F16 = mybir.dt.float16
AF = mybir.ActivationFunctionType
MULT = mybir.AluOpType.mult

S = 2048
DIM = 1024
HD = 64          # head dim
N_CORES = 8
NCH = DIM // 128  # 8 contraction chunks of 128


def build_kernel():
    nc = bacc.Bacc("TRN2", target_bir_lowering=False, debug=False, num_devices=N_CORES)

    xt_d = nc.dram_tensor("xt", [128, 4, NCH, 512], F16, kind="ExternalInput").ap()
    wq_d = nc.dram_tensor("wq", [DIM, 128], F16, kind="ExternalInput").ap()
    bq_d = nc.dram_tensor("bq", [128, 1], F32, kind="ExternalInput").ap()
    wkv_d = nc.dram_tensor("wkv", [DIM, 128], F16, kind="ExternalInput").ap()
    bkv_d = nc.dram_tensor("bkv", [128, 1], F32, kind="ExternalInput").ap()
    o_d = nc.dram_tensor("o", [2, S, HD], F16, kind="ExternalOutput").ap()

    with tile.TileContext(nc) as tc:
        with (
            tc.tile_pool(name="const", bufs=1) as const_pool,
            tc.tile_pool(name="persist", bufs=1) as persist_pool,
            tc.tile_pool(name="pt", bufs=3) as pt_pool,
            tc.tile_pool(name="outs", bufs=2) as out_pool,
            tc.tile_pool(name="ps_big", bufs=3, space="PSUM") as ps_big,
            tc.tile_pool(name="ps_o", bufs=2, space="PSUM") as ps_o,
        ):
            # ---- x stream: one single-descriptor DMA per block (sync HWDGE);
            # block qb0 split in halves so the KV projection starts sooner
            xT = persist_pool.tile([128, 4, NCH, 512], F16)   # 4 MB
            nc.sync.dma_start(xT[:, 0, 0:4, :], xt_d[:, 0, 0:4, :])
            nc.sync.dma_start(xT[:, 0, 4:8, :], xt_d[:, 0, 4:8, :])
            for qb in range(1, 4):
                nc.sync.dma_start(xT[:, qb], xt_d[:, qb])

            # ---- weights + biases on the Activation HWDGE queue (runs before
            # any exp), transfers overlap the sync queue's x stream
            wkv_sb = const_pool.tile([128, NCH, 128], F16)
            wq_sb = const_pool.tile([128, NCH, 128], F16)
            bq_sb = const_pool.tile([128, 1], F32)
            bkv_sb = const_pool.tile([128, 1], F32)
            nc.scalar.dma_start(wkv_sb[:], wkv_d.rearrange("(c p) d -> p c d", p=128))
            nc.scalar.dma_start(wq_sb[:], wq_d.rearrange("(c p) d -> p c d", p=128))
            # biases ride the gpsimd software queue: tiny transfers that must
            # NOT queue behind the 4.5MB x/weights stream on the HWDGE engines
            nc.gpsimd.dma_start(bkv_sb[:], bkv_d[:])
            nc.gpsimd.dma_start(bq_sb[:], bq_d[:])

            # ---- small constants (gpsimd software queue) ----
            ident32 = const_pool.tile([128, 128], F32)
            make_identity(nc, ident32[:])
            ident16 = const_pool.tile([128, 128], F16)
            nc.vector.tensor_copy(ident16[:], ident32[:])
            # I_64 in partitions 64..127 for base-64 V transposes
            ident2 = const_pool.tile([128, 64], F16)
            nc.vector.tensor_copy(ident2[0:64, :], ident16[0:64, 0:64])
            nc.gpsimd.dma_start(ident2[64:128, :], ident2[0:64, :])

            # ---- persistent SBUF tensors ----
            qt_sb = persist_pool.tile([128, S], F16)       # heads packed: h*64+d
            kv_sb = persist_pool.tile([128, S], F16)       # rows 0:64 KT, 64:128 VT
            kt2 = persist_pool.tile([128, S], F16)         # rows 64:128 = KT (for h1)
            v_sb = persist_pool.tile([128, 16, 65], F16)   # V' chunks (+ones col)
            nc.gpsimd.memset(v_sb[:, :, 64:65], 1.0)

            # ---- projection / setup helpers ----
            proj_tiles = {}

            def proj_chunks(w_sb, key, qb, cs):
                if cs[0] == 0:
                    proj_tiles[key] = ps_big.tile(
                        [128, 512], F32, tag="big", name="proj")
                pt_ = proj_tiles[key]
                for c in cs:
                    nc.tensor.matmul(pt_[:], w_sb[:, c, :], xT[:, qb, c, :],
                                     start=(c == 0), stop=(c == NCH - 1))

            def kv_drain(key, qb):
                sl = slice(qb * 512, (qb + 1) * 512)
                nc.vector.tensor_scalar_add(kv_sb[:, sl], proj_tiles[key][:],
                                            bkv_sb[:])
                # KT rows for head 1 live at partitions 64.. (DMA shifts partitions)
                nc.gpsimd.dma_start(kt2[64:128, sl], kv_sb[0:64, sl])

            def q_drain(key, qb):
                sl = slice(qb * 512, (qb + 1) * 512)
                nc.vector.tensor_scalar_add(qt_sb[:, sl], proj_tiles[key][:],
                                            bq_sb[:])

            def v_transpose4(kb0):
                """Transpose V chunks kb0..kb0+3 (one PSUM tile, one DVE copy)."""
                vt = ps_big.tile([128, 4, 66], F16, tag="big", name="vt")
                for j in range(4):
                    kb = kb0 + j
                    nc.tensor.matmul(
                        vt[:, j, 0:64], kv_sb[64:128, kb * 128:(kb + 1) * 128],
                        ident2[64:128, :], is_transpose=True)
                nc.vector.tensor_copy(v_sb[:, kb0:kb0 + 4, 0:64], vt[:, :, 0:64])

            # ---- attention round machinery ----
            class Group:
                def __init__(self, h, qb):
                    self.h, self.qb = h, qb
                    self.hb = h * HD
                    self.qsl = slice(qb * 512, (qb + 1) * 512)
                    self.pso = None
                    self.pss = {}
                    self.pts = {}

            def scores_exp(grp, r):
                pss = ps_big.tile([128, 1024], F32, tag="big", name="pss")
                grp.pss[r] = pss
                for u in range(2):
                    kb = r * 2 + u
                    if grp.h == 0:
                        lhs = kv_sb[0:64, kb * 128:(kb + 1) * 128]
                    else:
                        lhs = kt2[64:128, kb * 128:(kb + 1) * 128]
                    nc.tensor.matmul(
                        pss[:, u * 512:(u + 1) * 512], lhs,
                        qt_sb[grp.hb:grp.hb + HD, grp.qsl], start=True, stop=True)
                ptile = pt_pool.tile([128, 1024], F16, name="pt")
                grp.pts[r] = ptile
                nc.scalar.activation(ptile[:], pss[:], AF.Exp)

            def pv(grp, r):
                if grp.pso is None:
                    grp.pso = ps_o.tile([65, 512], F32, tag="ot", name="pso")
                for u in range(2):
                    kb = r * 2 + u
                    nc.tensor.matmul(
                        grp.pso[:], v_sb[:, kb, :],
                        grp.pts[r][:, u * 512:(u + 1) * 512],
                        start=(kb == 0), stop=(kb == 15),
                        skip_group_check=True)
                del grp.pts[r], grp.pss[r]

            def drain(grp):
                # copy O^T to SBUF fp16, PE-transpose back to s-major (tile
                # shares the big tag/ring), batched reciprocal of the four
                # denominator columns, scale, fp16 output DMA.
                ot_sb = out_pool.tile([65, 512], F16, tag="ot_sb")
                nc.vector.tensor_copy(ot_sb[:], grp.pso[:])
                tr = ps_big.tile([128, 4, 66], F16, tag="big", name="trd")
                for j in range(4):
                    nc.tensor.transpose(
                        tr[:, j, 0:65], ot_sb[:, j * 128:(j + 1) * 128],
                        ident16[0:65, 0:65])
                rcp = out_pool.tile([128, 4], F32, tag="rcp")
                nc.vector.reciprocal(rcp[:], tr[:, :, 64])
                o_sb = out_pool.tile([128, 4, HD], F16, tag="o_sb")
                for j in range(4):
                    nc.vector.tensor_scalar_mul(
                        o_sb[:, j, :], tr[:, j, 0:64], rcp[:, j:j + 1])
                nc.gpsimd.dma_start(
                    o_d[grp.h, grp.qsl, :].rearrange("(t j) c -> j t c", j=128),
                    o_sb[:])

            # ---- pre-fill: KV(qb0) chasing the two half-DMAs, V'(0..3), Q(qb0)
            proj_chunks(wkv_sb, "kv0", 0, [0, 1, 2, 3])
            proj_chunks(wkv_sb, "kv0", 0, [4, 5, 6, 7])
            kv_drain("kv0", 0)
            v_transpose4(0)
            proj_chunks(wq_sb, "q0", 0, list(range(NCH)))
            q_drain("q0", 0)

            # ---- item list: (group, round); A/B interleaved to track the
            # xT DMA stream, then the remaining 6 groups sequential ----
            A = Group(0, 0)
            B = Group(1, 0)
            C, D, E, F, G, H = (Group(0, 1), Group(1, 1), Group(0, 2),
                                Group(1, 2), Group(0, 3), Group(1, 3))
            items = ([(A, 0), (A, 1), (B, 0), (B, 1), (A, 2), (A, 3),
                      (B, 2), (B, 3), (A, 4), (A, 5), (B, 4), (B, 5),
                      (A, 6), (A, 7), (B, 6), (B, 7)]
                     + [(grp, r) for grp in (C, D, E, F, G, H)
                        for r in range(8)])

            # fillers: pre[i] runs before S(item i) (producers, program order
            # = dataflow). All projection work lands in the DMA-gated fill.
            pre = {i: [] for i in range(len(items))}
            pre[3].append(lambda: proj_chunks(wkv_sb, "kv1", 1, [0, 1, 2, 3]))
            pre[4].append(lambda: (proj_chunks(wkv_sb, "kv1", 1, [4, 5, 6, 7]),
                                   kv_drain("kv1", 1)))
            pre[5].append(lambda: (v_transpose4(4),
                                   proj_chunks(wq_sb, "q1", 1, [0, 1, 2, 3])))
            pre[6].append(lambda: (proj_chunks(wq_sb, "q1", 1, [4, 5, 6, 7]),
                                   q_drain("q1", 1)))
            pre[7].append(lambda: proj_chunks(wkv_sb, "kv2", 2, [0, 1, 2, 3]))
            pre[8].append(lambda: (proj_chunks(wkv_sb, "kv2", 2, [4, 5, 6, 7]),
                                   kv_drain("kv2", 2)))
            pre[9].append(lambda: (v_transpose4(8),
                                   proj_chunks(wq_sb, "q2", 2, [0, 1, 2, 3])))
            pre[10].append(lambda: (proj_chunks(wq_sb, "q2", 2, [4, 5, 6, 7]),
                                    q_drain("q2", 2)))
            pre[11].append(lambda: proj_chunks(wkv_sb, "kv3", 3, [0, 1, 2, 3]))
            pre[12].append(lambda: (proj_chunks(wkv_sb, "kv3", 3, [4, 5, 6, 7]),
                                    kv_drain("kv3", 3)))
            pre[13].append(lambda: (v_transpose4(12),
                                    proj_chunks(wq_sb, "q3", 3, [0, 1, 2, 3])))
            pre[14].append(lambda: (proj_chunks(wq_sb, "q3", 3, [4, 5, 6, 7]),
                                    q_drain("q3", 3)))

            # ---- flat software pipeline ----
            n = len(items)
            for f in pre[0]:
                f()
            scores_exp(*items[0])
            for i in range(n):
                if i + 1 < n:
                    for f in pre[i + 1]:
                        f()
                    scores_exp(*items[i + 1])
                grp, r = items[i]
                pv(grp, r)
                if r == 7:
                    drain(grp)

    nc.compile()
    return nc


_NC_CACHE = None


def make_in_maps(inputs):
    x = np.asarray(inputs["x"], np.float32).reshape(S, DIM)
    # [dim, s] -> [p][qb][c][t] with dim = c*128+p, s = qb*512+t
    xt = np.ascontiguousarray(
        x.T.reshape(NCH, 128, 4, 512).transpose(1, 2, 0, 3)).astype(np.float16)
    Wq = np.asarray(inputs["Wq"], np.float32)
    bq = np.asarray(inputs["bq"], np.float32)
    Wk = np.asarray(inputs["Wk"], np.float32)
    bk = np.asarray(inputs["bk"], np.float32)
    Wv = np.asarray(inputs["Wv"], np.float32)
    bv = np.asarray(inputs["bv"], np.float32)

    in_maps = []
    for d in range(N_CORES):
        g = d // 2
        wkv = np.concatenate(
            [Wk[:, g * 64:(g + 1) * 64], Wv[:, g * 64:(g + 1) * 64]], axis=1)
        bkv = np.concatenate([bk[g * 64:(g + 1) * 64], bv[g * 64:(g + 1) * 64]])
        in_maps.append({
            "xt": xt,
            "wq": (np.ascontiguousarray(Wq[:, d * 128:(d + 1) * 128]) / 8.0
                   ).astype(np.float16),
            "bq": (bq[d * 128:(d + 1) * 128] / 8.0).reshape(128, 1),
            "wkv": np.ascontiguousarray(wkv).astype(np.float16),
            "bkv": bkv.reshape(128, 1).copy(),
        })
    return in_maps


def kernel(**inputs) -> np.ndarray:
    global _NC_CACHE
    if _NC_CACHE is None:
        _NC_CACHE = build_kernel()
    nc = _NC_CACHE
    in_maps = make_in_maps(inputs)
    res = run_bass_kernel_spmd(nc, in_maps, list(range(N_CORES)))
    blocks = [np.asarray(res.results[d]["o"]).astype(np.float32).reshape(256, DIM)
              for d in range(N_CORES)]
    return np.concatenate(blocks, axis=0).reshape(1, S, DIM)


# revision 23
# speedup vs baseline: 1.4625x; 1.0879x over previous
"""GQA kernel for Trainium2 (Bass/Tile), 8-core head-parallel. v4.

Problem: x(1,2048,1024), Wq(1024,1024)+bq, Wk/Wv(1024,256)+bk/bv,
16 Q heads / 4 KV heads, head_dim 64, full (non-causal) softmax attention.
Reference output is attn(B,H,S,Dh) reshaped DIRECTLY to (B,S,H*Dh):
core d owns Q heads {2d, 2d+1} (one KV group d//2), producing a contiguous
(256,1024) slab of the final output; gather = concat.

The kernel is ACT-bound: 64 exp instructions of [128,1024] (~1.11us each)
are the critical path. Scheduling keeps the Scalar engine busy on exp from
~6us onward:
- fp16 everywhere (same PE speed as bf16, 10x better accuracy).
- flat software-pipelined item list: item = one attention round
  (2 scores mm -> exp -> 2 PV mm). S(i+1) is emitted before PV(i); a
  3-deep scores ring keeps S(i+2) off the exp critical path.
- first 16 items interleave groups (0,0) and (1,0) so early rounds only
  need early xT blocks while qb2/qb3 still stream from HBM.
- xt DRAM layout is [p][qb][c][s-block] so each xT block DMA is a single
  contiguous descriptor per partition (HWDGE trigger is cheap); weights
  ride the Activation engine's HWDGE queue, parallel to the sync queue.
- KV/Q projections and V' transposes are fillers inside the DMA-gated
  fill window, placed just-in-time before their first consumer.
- output drain avoids PSUM/PE entirely: reciprocal of the denominator row
  (DVE) -> partition_broadcast (Pool) -> tensor_tensor multiply (DVE) ->
  4 DMA-XBAR transposes (sync HWDGE) -> fp16 output DMA (host casts).
- PSUM: 3x scores[128,1024] (tag also hosts proj + V'-transpose tiles)
  + 2x pso[65,512] = 8 banks.
"""

import numpy as np

import concourse.bass as bass
import concourse.mybir as mybir
import concourse.tile as tile
from concourse import bacc
from concourse.bass_utils import run_bass_kernel_spmd
from concourse.masks import make_identity

F32 = mybir.dt.float32
F16 = mybir.dt.float16
AF = mybir.ActivationFunctionType
MULT = mybir.AluOpType.mult

S = 2048
DIM = 1024
HD = 64          # head dim
N_CORES = 8
NCH = DIM // 128  # 8 contraction chunks of 128


def build_kernel():
    nc = bacc.Bacc("TRN2", target_bir_lowering=False, debug=False, num_devices=N_CORES)

    xt_d = nc.dram_tensor("xt", [128, 4, NCH, 512], F16, kind="ExternalInput").ap()
    wq_d = nc.dram_tensor("wq", [DIM, 128], F16, kind="ExternalInput").ap()
    bq_d = nc.dram_tensor("bq", [128, 1], F32, kind="ExternalInput").ap()
    wkv_d = nc.dram_tensor("wkv", [DIM, 128], F16, kind="ExternalInput").ap()
    bkv_d = nc.dram_tensor("bkv", [128, 1], F32, kind="ExternalInput").ap()
    o_d = nc.dram_tensor("o", [2, 4, 4, 128, HD], F16, kind="ExternalOutput").ap()

    with tile.TileContext(nc) as tc:
        with (
            tc.tile_pool(name="const", bufs=1) as const_pool,
            tc.tile_pool(name="persist", bufs=1) as persist_pool,
            tc.tile_pool(name="pt", bufs=3) as pt_pool,
            tc.tile_pool(name="outs", bufs=2) as out_pool,
            tc.tile_pool(name="ps_big", bufs=3, space="PSUM") as ps_big,
            tc.tile_pool(name="ps_o", bufs=2, space="PSUM") as ps_o,
        ):
            # ---- x stream: one single-descriptor DMA per block (sync HWDGE);
            # block qb0 split in halves so the KV projection starts sooner
            # weights first (small, gate the first projections), then the x
            # stream; all on the sync HWDGE queue so completion follows
            # trigger order
            wkv_sb = const_pool.tile([128, NCH, 128], F16)
            wq_sb = const_pool.tile([128, NCH, 128], F16)
            bq_sb = const_pool.tile([128, 1], F32)
            bkv_sb = const_pool.tile([128, 1], F32)
            nc.sync.dma_start(wkv_sb[:], wkv_d.rearrange("(c p) d -> p c d", p=128))
            nc.sync.dma_start(wq_sb[:], wq_d.rearrange("(c p) d -> p c d", p=128))
            xT = persist_pool.tile([128, 4, NCH, 512], F16)   # 4 MB
            nc.sync.dma_start(xT[:, 0, 0:4, :], xt_d[:, 0, 0:4, :])
            nc.sync.dma_start(xT[:, 0, 4:8, :], xt_d[:, 0, 4:8, :])
            for qb in range(1, 4):
                nc.sync.dma_start(xT[:, qb], xt_d[:, qb])
            # biases ride the gpsimd software queue: tiny transfers that must
            # NOT queue behind the 4.5MB x/weights stream on the HWDGE engines
            nc.gpsimd.dma_start(bkv_sb[:], bkv_d[:])
            nc.gpsimd.dma_start(bq_sb[:], bq_d[:])

            # ---- small constants (gpsimd software queue) ----
            ident32 = const_pool.tile([128, 128], F32)
            make_identity(nc, ident32[:])
            ident16 = const_pool.tile([128, 128], F16)
            nc.vector.tensor_copy(ident16[:], ident32[:])
            # I_64 in partitions 64..127 for base-64 V transposes
            ident2 = const_pool.tile([128, 64], F16)
            nc.vector.tensor_copy(ident2[0:64, :], ident16[0:64, 0:64])
            nc.gpsimd.dma_start(ident2[64:128, :], ident2[0:64, :])

            # ---- persistent SBUF tensors ----
            qt_sb = persist_pool.tile([128, S], F16)       # heads packed: h*64+d
            kv_sb = persist_pool.tile([128, S], F16)       # rows 0:64 KT, 64:128 VT
            kt2 = persist_pool.tile([128, S], F16)         # rows 64:128 = KT (for h1)
            v_sb = persist_pool.tile([128, 16, 65], F16)   # V' chunks (+ones col)
            nc.gpsimd.memset(v_sb[:, :, 64:65], 1.0)

            # ---- projection / setup helpers ----
            proj_tiles = {}

            def proj_chunks(w_sb, key, qb, cs):
                if cs[0] == 0:
                    proj_tiles[key] = ps_big.tile(
                        [128, 512], F32, tag="big", name="proj")
                pt_ = proj_tiles[key]
                for c in cs:
                    nc.tensor.matmul(pt_[:], w_sb[:, c, :], xT[:, qb, c, :],
                                     start=(c == 0), stop=(c == NCH - 1))

            def kv_drain(key, qb):
                sl = slice(qb * 512, (qb + 1) * 512)
                nc.vector.tensor_scalar_add(kv_sb[:, sl], proj_tiles[key][:],
                                            bkv_sb[:])
                # KT rows for head 1 live at partitions 64.. (DMA shifts partitions)
                nc.gpsimd.dma_start(kt2[64:128, sl], kv_sb[0:64, sl])

            def q_drain(key, qb):
                sl = slice(qb * 512, (qb + 1) * 512)
                nc.vector.tensor_scalar_add(qt_sb[:, sl], proj_tiles[key][:],
                                            bq_sb[:])

            def v_transpose4(kb0):
                """Transpose V chunks kb0..kb0+3 (one PSUM tile, one DVE copy)."""
                vt = ps_big.tile([128, 4, 66], F16, tag="big", name="vt")
                for j in range(4):
                    kb = kb0 + j
                    nc.tensor.matmul(
                        vt[:, j, 0:64], kv_sb[64:128, kb * 128:(kb + 1) * 128],
                        ident2[64:128, :], is_transpose=True)
                nc.vector.tensor_copy(v_sb[:, kb0:kb0 + 4, 0:64], vt[:, :, 0:64])

            # ---- attention round machinery ----
            class Group:
                def __init__(self, h, qb):
                    self.h, self.qb = h, qb
                    self.hb = h * HD
                    self.qsl = slice(qb * 512, (qb + 1) * 512)
                    self.pso = None
                    self.pss = {}
                    self.pts = {}

            def scores_exp(grp, r):
                pss = ps_big.tile([128, 1024], F32, tag="big", name="pss")
                grp.pss[r] = pss
                for u in range(2):
                    kb = r * 2 + u
                    if grp.h == 0:
                        lhs = kv_sb[0:64, kb * 128:(kb + 1) * 128]
                    else:
                        lhs = kt2[64:128, kb * 128:(kb + 1) * 128]
                    nc.tensor.matmul(
                        pss[:, u * 512:(u + 1) * 512], lhs,
                        qt_sb[grp.hb:grp.hb + HD, grp.qsl], start=True, stop=True)
                ptile = pt_pool.tile([128, 1024], F16, name="pt")
                grp.pts[r] = ptile
                nc.scalar.activation(ptile[:], pss[:], AF.Exp)

            def pv(grp, r):
                if grp.pso is None:
                    grp.pso = ps_o.tile([65, 512], F32, tag="ot", name="pso")
                for u in range(2):
                    kb = r * 2 + u
                    nc.tensor.matmul(
                        grp.pso[:], v_sb[:, kb, :],
                        grp.pts[r][:, u * 512:(u + 1) * 512],
                        start=(kb == 0), stop=(kb == 15),
                        skip_group_check=True)
                del grp.pts[r], grp.pss[r]

            def drain_part1(grp):
                # copy O^T to SBUF fp16, PE-transpose back to s-major (tile
                # shares the big tag/ring). Deferred one item past the last
                # PV so the transposes land in PE wait-windows, not in the
                # scores->exp critical path.
                grp.ot_sb = out_pool.tile([65, 512], F16, tag="ot_sb")
                nc.vector.tensor_copy(grp.ot_sb[:], grp.pso[:])
                grp.tr = ps_big.tile([128, 4, 66], F16, tag="big", name="trd")
                for j in range(4):
                    nc.tensor.transpose(
                        grp.tr[:, j, 0:65], grp.ot_sb[:, j * 128:(j + 1) * 128],
                        ident16[0:65, 0:65])

            def drain_part2(grp):
                rcp = out_pool.tile([128, 4], F32, tag="rcp")
                nc.vector.reciprocal(rcp[:], grp.tr[:, :, 64])
                o_sb = out_pool.tile([128, 4, HD], F16, tag="o_sb")
                for j in range(4):
                    nc.vector.tensor_scalar_mul(
                        o_sb[:, j, :], grp.tr[:, j, 0:64], rcp[:, j:j + 1])
                nc.sync.dma_start(
                    o_d[grp.h, grp.qb].rearrange("t j c -> j t c"), o_sb[:])

            # ---- pre-fill: KV(qb0) chasing the two half-DMAs, V'(0..3), Q(qb0)
            proj_chunks(wkv_sb, "kv0", 0, [0, 1, 2, 3])
            proj_chunks(wkv_sb, "kv0", 0, [4, 5, 6, 7])
            kv_drain("kv0", 0)
            v_transpose4(0)
            proj_chunks(wq_sb, "q0", 0, list(range(NCH)))
            q_drain("q0", 0)

            # ---- item list: (group, round); A/B interleaved to track the
            # xT DMA stream, then the remaining 6 groups sequential ----
            A = Group(0, 0)
            B = Group(1, 0)
            C, D, E, F, G, H = (Group(0, 1), Group(1, 1), Group(0, 2),
                                Group(1, 2), Group(0, 3), Group(1, 3))
            items = ([(A, 0), (A, 1), (B, 0), (B, 1), (A, 2), (A, 3),
                      (B, 2), (B, 3), (A, 4), (A, 5), (B, 4), (B, 5),
                      (A, 6), (A, 7), (B, 6), (B, 7)]
                     + [(grp, r) for grp in (C, D, E, F, G, H)
                        for r in range(8)])

            # fillers: pre[i] runs before S(item i) (producers, program order
            # = dataflow). All projection work lands in the DMA-gated fill.
            pre = {i: [] for i in range(len(items))}
            pre[3].append(lambda: proj_chunks(wkv_sb, "kv1", 1, [0, 1, 2, 3]))
            pre[4].append(lambda: (proj_chunks(wkv_sb, "kv1", 1, [4, 5, 6, 7]),
                                   kv_drain("kv1", 1)))
            pre[5].append(lambda: (v_transpose4(4),
                                   proj_chunks(wq_sb, "q1", 1, [0, 1, 2, 3])))
            pre[6].append(lambda: (proj_chunks(wq_sb, "q1", 1, [4, 5, 6, 7]),
                                   q_drain("q1", 1)))
            pre[7].append(lambda: proj_chunks(wkv_sb, "kv2", 2, [0, 1, 2, 3]))
            pre[8].append(lambda: (proj_chunks(wkv_sb, "kv2", 2, [4, 5, 6, 7]),
                                   kv_drain("kv2", 2)))
            pre[9].append(lambda: (v_transpose4(8),
                                   proj_chunks(wq_sb, "q2", 2, [0, 1, 2, 3])))
            pre[10].append(lambda: (proj_chunks(wq_sb, "q2", 2, [4, 5, 6, 7]),
                                    q_drain("q2", 2)))
            pre[11].append(lambda: proj_chunks(wkv_sb, "kv3", 3, [0, 1, 2, 3]))
            pre[12].append(lambda: (proj_chunks(wkv_sb, "kv3", 3, [4, 5, 6, 7]),
                                    kv_drain("kv3", 3)))
            pre[13].append(lambda: (v_transpose4(12),
                                    proj_chunks(wq_sb, "q3", 3, [0, 1, 2, 3])))
            pre[14].append(lambda: (proj_chunks(wq_sb, "q3", 3, [4, 5, 6, 7]),
                                    q_drain("q3", 3)))

            # ---- flat software pipeline (drains deferred/split) ----
            n = len(items)
            deferred = {}
            for f in pre[0]:
                f()
            scores_exp(*items[0])
            for i in range(n):
                if i + 1 < n:
                    for f in pre[i + 1]:
                        f()
                    scores_exp(*items[i + 1])
                for f in deferred.pop(i, []):
                    f(
                    )
                grp, r = items[i]
                pv(grp, r)
                if r == 7:
                    deferred.setdefault(i + 1, []).append(
                        lambda g=grp: drain_part1(g))
                    deferred.setdefault(i + 2, []).append(
                        lambda g=grp: drain_part2(g))
            for i in sorted(deferred):
                for f in deferred[i]:
                    f()

    nc.compile()
    return nc


_NC_CACHE = None


def make_in_maps(inputs):
    x = np.asarray(inputs["x"], np.float32).reshape(S, DIM)
    # [dim, s] -> [p][qb][c][t] with dim = c*128+p, s = qb*512+t
    xt = np.ascontiguousarray(
        x.T.reshape(NCH, 128, 4, 512).transpose(1, 2, 0, 3)).astype(np.float16)
    Wq = np.asarray(inputs["Wq"], np.float32)
    bq = np.asarray(inputs["bq"], np.float32)
    Wk = np.asarray(inputs["Wk"], np.float32)
    bk = np.asarray(inputs["bk"], np.float32)
    Wv = np.asarray(inputs["Wv"], np.float32)
    bv = np.asarray(inputs["bv"], np.float32)

    in_maps = []
    for d in range(N_CORES):
        g = d // 2
        wkv = np.concatenate(
            [Wk[:, g * 64:(g + 1) * 64], Wv[:, g * 64:(g + 1) * 64]], axis=1)
        bkv = np.concatenate([bk[g * 64:(g + 1) * 64], bv[g * 64:(g + 1) * 64]])
        in_maps.append({
            "xt": xt,
            "wq": (np.ascontiguousarray(Wq[:, d * 128:(d + 1) * 128]) / 8.0
                   ).astype(np.float16),
            "bq": (bq[d * 128:(d + 1) * 128] / 8.0).reshape(128, 1),
            "wkv": np.ascontiguousarray(wkv).astype(np.float16),
            "bkv": bkv.reshape(128, 1).copy(),
        })
    return in_maps


def kernel(**inputs) -> np.ndarray:
    global _NC_CACHE
    if _NC_CACHE is None:
        _NC_CACHE = build_kernel()
    nc = _NC_CACHE
    in_maps = make_in_maps(inputs)
    res = run_bass_kernel_spmd(nc, in_maps, list(range(N_CORES)))
    blocks = [np.asarray(res.results[d]["o"]).astype(np.float32).reshape(256, DIM)
              for d in range(N_CORES)]
    return np.concatenate(blocks, axis=0).reshape(1, S, DIM)
